# revision 14
# baseline (speedup 1.0000x reference)
"""Trainium2 Bass kernel for the CrossEntropyMap loss.

Math (per batch row b of y_hat[B=64, T=64, G=128, G]):
    lse_b  = logsumexp(y_hat[b].reshape(-1))            # over T*G*G = 1M classes
    pick_b = sum_t y_hat[b, t, xi[b,t], yi[b,t]]        # xi/yi = round(coords*G)
    loss   = mean_b(T * lse_b - pick_b)

Sharding: data-parallel over batch, 8 rows per NeuronCore (32 MiB/core).

Device kernel (per core): stream the 8 rows as 16 half-row [128, 4096] chunks
and run one ACT pass per chunk: exp(x + C_SHIFT) with accum_out giving the
per-partition sums S[p, c]. The constant shift is mathematically exact for
logsumexp (it only scales the partial sums); C_SHIFT=-16 keeps exp in range
for |x| up to ~100. The 512 picked logits are gathered with indirect DMAs
(f32, straight from HBM). One PE matmul with a ones vector reduces
[S | -picksum] over partitions to a [17, 1] output per core; the host folds
ln(), the shift and the batch mean while unsharding (64 scalar lns total).

DMA strategy: a single DGE queue only sustains ~210-240 GB/s, but with both
HWDGE rings (sync = qSPDynamicHW, scalar = qActDynamicHW) streaming
concurrently the 16 SDMA engines saturate at ~425 GB/s aggregate (measured;
~26.6 GB/s per SBUF AXI port). Chunks alternate between the two rings with
EQUAL bytes so both stay busy until the very end, and exps consume them in
the same alternating order so ACT tracks arrivals without head-of-line
blocking. A third SWDGE bulk stream or f32->bf16 in-flight cast does NOT
raise the ceiling (same 16 engines bind on the read side) — measured 425
GB/s either way — so gpsimd only runs the pick gather.
"""

import sys

import numpy as np

try:
    import concourse.bacc as bacc
except ImportError:  # pragma: no cover - fallback for bare environments
    sys.path.insert(0, "/opt/trn_rl_repo")
    import concourse.bacc as bacc

import concourse.bass as bass
import concourse.tile as tile
from concourse import mybir
from concourse.bass_utils import run_bass_kernel_spmd

B, T, G = 64, 64, 128
N_CORES = 8
ROWS = B // N_CORES            # 8 batch rows per core
ROW_ELEMS = T * G * G          # 1_048_576 classes per row
P = 128
F = ROW_ELEMS // P             # 8192 elements per partition per row
HALVES = 2                     # chunks per row
FH = F // HALVES               # 4096 per chunk
N_CHUNKS = ROWS * HALVES       # 16
N_PER_CORE = ROWS * ROW_ELEMS  # 8_388_608 elements per core shard
PICKS = ROWS * T               # 512 gathered logits per core
PICK_F = PICKS // P            # 4 per partition
C_SHIFT = -16.0                # constant exp bias (exact for logsumexp)

_f32 = mybir.dt.float32
_bf16 = mybir.dt.bfloat16
_i32 = mybir.dt.int32
_EXP = mybir.ActivationFunctionType.Exp
_AXF = mybir.AxisListType.X
_ADD = mybir.AluOpType.add

# --- stream configuration ---------------------------------------------------
# 'sy' = sync HWDGE ring, 'sc' = scalar HWDGE ring. Measured: the 16 SDMA
# engines cap at ~425 GB/s aggregate (~26.6 GB/s per port) once >=2
# descriptor streams are in flight, while a single stream only sustains
# ~210-240 GB/s — so split the bytes EQUALLY between both HWDGE rings and
# keep both busy until the very end. A third SWDGE bulk stream does not
# raise the ceiling (same engines), so gpsimd only runs the pick gather.
#
# Chunk list: (row, eighth_start, n_eighths) in units of F/8 = 1024 elems
# per partition. Rows 0-6 stream as half-row 2 MiB chunks; row 7 tapers
# (2x 1 MiB then 4x 0.5 MiB) so the final exps are short and the two queues'
# simultaneous drain doesn't leave a serialized 2x3.7us exp tail.
CHUNKS = [(c // 2, (c % 2) * 4, 4) for c in range(14)]          # rows 0-6
CHUNKS += [(7, 0, 2), (7, 2, 2), (7, 4, 1), (7, 5, 1), (7, 6, 1), (7, 7, 1)]
N_CHUNKS_DEV = len(CHUNKS)                                      # 20
CHUNK_STREAM = ["sy", "sc"] * (N_CHUNKS_DEV // 2)
# exp consumption order = arrival order (queues deliver alternately).
EXP_ORDER = list(range(N_CHUNKS_DEV))
# host mapping: per-row list of device columns to sum for that row's S
ROW_COLS = [[2 * r, 2 * r + 1] for r in range(7)] + [[14, 15, 16, 17, 18, 19]]
PREFILL = {"sy": 3, "sc": 3}

_compiled_nc = None
LAST_RESULTS = None  # test hook: BassKernelResults of the last run


def build_nc():
    nc = bacc.Bacc("TRN2", target_bir_lowering=False, debug=False)
    y = nc.dram_tensor("y", [N_PER_CORE, 1], _f32, kind="ExternalInput")
    idx = nc.dram_tensor("idx", [P, PICK_F], _i32, kind="ExternalInput")
    out = nc.dram_tensor("out", [N_CHUNKS_DEV + 1, 1], _f32, kind="ExternalOutput")

    # chunk views at half/quarter/eighth-row granularity: partition p of row r
    # holds elements [r*1M + p*8192, +8192) — contiguous per partition, so any
    # run of eighths (1024 elems) is one contiguous span per partition.
    y_view = {
        4: y.ap().rearrange("(r p h f) o -> r h p (f o)", r=ROWS, p=P, h=2),
        2: y.ap().rearrange("(r p q f) o -> r q p (f o)", r=ROWS, p=P, q=4),
        1: y.ap().rearrange("(r p e f) o -> r e p (f o)", r=ROWS, p=P, e=8),
    }

    def chunk_ap(c):
        r, e0, n = CHUNKS[c]
        assert e0 % n == 0
        return y_view[n][r, e0 // n]

    with tile.TileContext(nc) as tc:
        with (
            tc.tile_pool(name="xpool", bufs=sum(PREFILL.values())) as xpool,
            tc.tile_pool(name="small", bufs=1) as small,
            tc.tile_pool(name="psum", bufs=1, space="PSUM") as psum,
        ):
            engines = {"sy": nc.sync, "sc": nc.scalar}

            ones = small.tile([P, 1], _f32)
            nc.vector.memset(ones[:], 1.0)
            cbias = small.tile([P, 1], _f32)
            nc.vector.memset(cbias[:], C_SHIFT)
            idx_sb = small.tile([P, PICK_F], _i32)
            nc.sync.dma_start(out=idx_sb[:], in_=idx.ap())

            # s_all[:, c] = per-partition sum of exp(chunk c); last col = -picksum
            s_all = small.tile([P, N_CHUNKS_DEV + 1], _f32)

            # per-stream chunk lists in consumption order
            stream_chunks = {s: [c for c in EXP_ORDER if CHUNK_STREAM[c] == s]
                             for s in ("sy", "sc")}
            next_issue = {s: 0 for s in stream_chunks}
            x_tiles = {}

            def issue_dma(s):
                i = next_issue[s]
                if i >= len(stream_chunks[s]):
                    return
                next_issue[s] = i + 1
                c = stream_chunks[s][i]
                w = CHUNKS[c][2] * (F // 8)
                xt = xpool.tile([P, FH], _f32, tag="x")
                engines[s].dma_start(out=xt[:, 0:w], in_=chunk_ap(c))
                x_tiles[c] = xt

            # prefill both rings, alternating so buffer rotation matches
            # consumption order
            for _ in range(PREFILL["sy"]):
                issue_dma("sy")
                issue_dma("sc")

            # picked-logit gather on the otherwise idle SWDGE queue; data is
            # only needed at the final reduce.
            picked = small.tile([P, PICK_F], _f32)
            for j in range(PICK_F):
                nc.gpsimd.indirect_dma_start(
                    out=picked[:, j : j + 1],
                    out_offset=None,
                    in_=y.ap(),
                    in_offset=bass.IndirectOffsetOnAxis(
                        ap=idx_sb[:, j : j + 1], axis=0
                    ),
                )
            # s_all[:, -1] = -sum_j picked[p, j]
            nc.vector.tensor_reduce(
                out=s_all[:, N_CHUNKS_DEV : N_CHUNKS_DEV + 1], in_=picked[:],
                axis=_AXF, op=_ADD, negate=True,
            )

            # stream the chunks through ACT in arrival order
            et = small.tile([P, FH], _bf16, tag="e")
            for c in EXP_ORDER:
                xt = x_tiles.pop(c)
                w = CHUNKS[c][2] * (F // 8)
                nc.scalar.activation(
                    out=et[:, 0:w], in_=xt[:, 0:w], func=_EXP,
                    bias=cbias[:, 0:1], scale=1.0,
                    accum_out=s_all[:, c : c + 1],
                )
                issue_dma(CHUNK_STREAM[c])

            # acc[j] = sum_p s_all[p, j]  (20 chunk sums + -picksum)
            acc = psum.tile([N_CHUNKS_DEV + 1, 1], _f32, tag="acc")
            nc.tensor.matmul(
                out=acc[:], lhsT=s_all[:], rhs=ones[:], start=True, stop=True
            )
            res = small.tile([N_CHUNKS_DEV + 1, 1], _f32)
            nc.vector.tensor_copy(out=res[:], in_=acc[:])
            nc.sync.dma_start(out=out.ap(), in_=res[:])

    nc.compile()
    return nc


def make_in_maps(y_hat: np.ndarray, coords: np.ndarray):
    """Shard inputs across cores and build per-core gather indices."""
    y_hat = np.ascontiguousarray(y_hat, dtype=np.float32)
    coords = np.asarray(coords, dtype=np.float32)

    # Match jnp.round (round-half-to-even); np.round has identical semantics,
    # and coords * 128 is exact in f32 (power-of-two scale).
    xi = np.round(coords[:, :, 0] * np.float32(G)).astype(np.int64)  # (B, T)
    yi = np.round(coords[:, :, 1] * np.float32(G)).astype(np.int64)  # (B, T)
    t = np.arange(T, dtype=np.int64)[None, :]
    flat = t * (G * G) + xi * G + yi  # (B, T) element offset within row b

    in_maps = []
    for c in range(N_CORES):
        rows = slice(c * ROWS, (c + 1) * ROWS)
        shard = y_hat[rows].reshape(N_PER_CORE, 1)
        local = np.arange(ROWS, dtype=np.int64)[:, None] * ROW_ELEMS + flat[rows]
        idx = local.reshape(P, PICK_F).astype(np.int32)
        in_maps.append({"y": shard, "idx": idx})
    return in_maps


def kernel(y_hat: np.ndarray, coords: np.ndarray) -> np.ndarray:
    global _compiled_nc, LAST_RESULTS
    in_maps = make_in_maps(y_hat, coords)
    if _compiled_nc is None:
        _compiled_nc = build_nc()
    res = run_bass_kernel_spmd(
        _compiled_nc, in_maps, core_ids=list(range(N_CORES))
    )
    LAST_RESULTS = res
    total = 0.0
    for r in res.results:
        v = np.asarray(r["out"]).reshape(-1).astype(np.float64)
        negpick = v[N_CHUNKS_DEV]
        s_rows = np.array([v[cols].sum() for cols in ROW_COLS])
        total += T * float(np.log(s_rows).sum()) + negpick
    loss = total / B + T * (-C_SHIFT)
    return np.array(np.float32(loss))


# revision 16
# speedup vs baseline: 1.1074x; 1.1074x over previous
"""Trainium2 Bass kernel for the CrossEntropyMap loss.

Math (per batch row b of y_hat[B=64, T=64, G=128, G]):
    lse_b  = logsumexp(y_hat[b].reshape(-1))            # over T*G*G = 1M classes
    pick_b = sum_t y_hat[b, t, xi[b,t], yi[b,t]]        # xi/yi = round(coords*G)
    loss   = mean_b(T * lse_b - pick_b)

Sharding: data-parallel over batch, 8 rows per NeuronCore (32 MiB/core).

Device kernel (per core): stream the 8 rows as 16 half-row [128, 4096] chunks
and run one ACT pass per chunk: exp(x + C_SHIFT) with accum_out giving the
per-partition sums S[p, c]. The constant shift is mathematically exact for
logsumexp (it only scales the partial sums); C_SHIFT=-16 keeps exp in range
for |x| up to ~100. The 512 picked logits are gathered with indirect DMAs
(f32, straight from HBM). One PE matmul with a ones vector reduces
[S | -picksum] over partitions to a [17, 1] output per core; the host folds
ln(), the shift and the batch mean while unsharding (64 scalar lns total).

DMA strategy: a single DGE queue only sustains ~210-240 GB/s, but with both
HWDGE rings (sync = qSPDynamicHW, scalar = qActDynamicHW) streaming
concurrently the 16 SDMA engines saturate at ~425 GB/s aggregate (measured;
~26.6 GB/s per SBUF AXI port). Chunks alternate between the two rings with
EQUAL bytes so both stay busy until the very end, and exps consume them in
the same alternating order so ACT tracks arrivals without head-of-line
blocking. A third SWDGE bulk stream or f32->bf16 in-flight cast does NOT
raise the ceiling (same 16 engines bind on the read side) — measured 425
GB/s either way — so gpsimd only runs the pick gather.
"""

import sys

import numpy as np

try:
    import concourse.bacc as bacc
except ImportError:  # pragma: no cover - fallback for bare environments
    sys.path.insert(0, "/opt/trn_rl_repo")
    import concourse.bacc as bacc

import concourse.bass as bass
import concourse.tile as tile
from concourse import mybir
from concourse.bass_utils import run_bass_kernel_spmd

B, T, G = 64, 64, 128
N_CORES = 8
ROWS = B // N_CORES            # 8 batch rows per core
ROW_ELEMS = T * G * G          # 1_048_576 classes per row
P = 128
F = ROW_ELEMS // P             # 8192 elements per partition per row
HALVES = 2                     # chunks per row
FH = F // HALVES               # 4096 per chunk
N_CHUNKS = ROWS * HALVES       # 16
N_PER_CORE = ROWS * ROW_ELEMS  # 8_388_608 elements per core shard
PICKS = ROWS * T               # 512 gathered logits per core
PICK_F = PICKS // P            # 4 per partition
C_SHIFT = -16.0                # constant exp bias (exact for logsumexp)

_f32 = mybir.dt.float32
_bf16 = mybir.dt.bfloat16
_i32 = mybir.dt.int32
_EXP = mybir.ActivationFunctionType.Exp
_AXF = mybir.AxisListType.X
_ADD = mybir.AluOpType.add

# --- stream configuration ---------------------------------------------------
# 'sy' = sync HWDGE ring, 'sc' = scalar HWDGE ring. Measured: the 16 SDMA
# engines cap at ~425 GB/s aggregate (~26.6 GB/s per port) once >=2
# descriptor streams are in flight, while a single stream only sustains
# ~210-240 GB/s — so split the bytes EQUALLY between both HWDGE rings and
# keep both busy until the very end. A third SWDGE bulk stream does not
# raise the ceiling (same engines), so gpsimd only runs the pick gather.
#
# Chunk list: (row, eighth_start, n_eighths) in units of F/8 = 1024 elems
# per partition. Row 0 tapers IN (2x 1 MiB then 2 MiB) so the first exp can
# start ~5us earlier — this matters in runs where ACT is p-state throttled
# (measured 1.0 GHz instead of 1.2 GHz) and becomes the critical path. Rows
# 1-6 stream as half-row 2 MiB chunks; row 7 tapers OUT (2x 1 MiB then
# 4x 0.5 MiB) so the two queues' simultaneous drain doesn't leave a
# serialized 2x3.7us exp tail. Queue assignment is balanced to exactly 16
# MiB per ring in near-alternating order.
CHUNKS = [(0, 0, 2), (0, 2, 2), (0, 4, 4)]
for _r in range(1, 7):
    CHUNKS += [(_r, 0, 4), (_r, 4, 4)]
CHUNKS += [(7, 0, 2), (7, 2, 2), (7, 4, 1), (7, 5, 1), (7, 6, 1), (7, 7, 1)]
N_CHUNKS_DEV = len(CHUNKS)                                      # 21
CHUNK_STREAM = ["sy", "sc", "sy", "sc", "sy", "sc", "sy", "sc", "sy", "sc",
                "sy", "sc", "sy", "sc", "sy", "sc", "sc", "sy", "sc", "sy",
                "sc"]
# exp consumption order = arrival order (queues deliver alternately).
EXP_ORDER = list(range(N_CHUNKS_DEV))
# host mapping: per-row list of device columns to sum for that row's S
ROW_COLS = ([[0, 1, 2]] + [[2 * r + 1, 2 * r + 2] for r in range(1, 7)]
            + [[15, 16, 17, 18, 19, 20]])
# deep prefill: 10 tiles in flight so most DMA dispatches run in the head
# (where ACT idles) instead of between exps on the ACT sequencer.
PREFILL = {"sy": 5, "sc": 5}

_compiled_nc = None
LAST_RESULTS = None  # test hook: BassKernelResults of the last run


def build_nc():
    nc = bacc.Bacc("TRN2", target_bir_lowering=False, debug=False)
    y = nc.dram_tensor("y", [N_PER_CORE, 1], _f32, kind="ExternalInput")
    idx = nc.dram_tensor("idx", [P, PICK_F], _i32, kind="ExternalInput")
    out = nc.dram_tensor("out", [N_CHUNKS_DEV + 1, 1], _f32, kind="ExternalOutput")

    # chunk views at half/quarter/eighth-row granularity: partition p of row r
    # holds elements [r*1M + p*8192, +8192) — contiguous per partition, so any
    # run of eighths (1024 elems) is one contiguous span per partition.
    y_view = {
        4: y.ap().rearrange("(r p h f) o -> r h p (f o)", r=ROWS, p=P, h=2),
        2: y.ap().rearrange("(r p q f) o -> r q p (f o)", r=ROWS, p=P, q=4),
        1: y.ap().rearrange("(r p e f) o -> r e p (f o)", r=ROWS, p=P, e=8),
    }

    def chunk_ap(c):
        r, e0, n = CHUNKS[c]
        assert e0 % n == 0
        return y_view[n][r, e0 // n]

    with tile.TileContext(nc) as tc:
        with (
            tc.tile_pool(name="xpool", bufs=sum(PREFILL.values())) as xpool,
            tc.tile_pool(name="small", bufs=1) as small,
            tc.tile_pool(name="psum", bufs=1, space="PSUM") as psum,
        ):
            engines = {"sy": nc.sync, "sc": nc.scalar}

            ones = small.tile([P, 1], _f32)
            nc.vector.memset(ones[:], 1.0)
            cbias = small.tile([P, 1], _f32)
            nc.vector.memset(cbias[:], C_SHIFT)
            idx_sb = small.tile([P, PICK_F], _i32)
            nc.sync.dma_start(out=idx_sb[:], in_=idx.ap())

            # s_all[:, c] = per-partition sum of exp(chunk c); last col = -picksum
            s_all = small.tile([P, N_CHUNKS_DEV + 1], _f32)

            # per-stream chunk lists in consumption order
            stream_chunks = {s: [c for c in EXP_ORDER if CHUNK_STREAM[c] == s]
                             for s in ("sy", "sc")}
            next_issue = {s: 0 for s in stream_chunks}
            x_tiles = {}

            def issue_dma(s):
                i = next_issue[s]
                if i >= len(stream_chunks[s]):
                    return
                next_issue[s] = i + 1
                c = stream_chunks[s][i]
                w = CHUNKS[c][2] * (F // 8)
                xt = xpool.tile([P, FH], _f32, tag="x")
                engines[s].dma_start(out=xt[:, 0:w], in_=chunk_ap(c))
                x_tiles[c] = xt

            # prefill both rings in global chunk order so buffer rotation
            # matches consumption order
            for c in range(PREFILL["sy"] + PREFILL["sc"]):
                issue_dma(CHUNK_STREAM[c])

            # picked-logit gather on the otherwise idle SWDGE queue; data is
            # only needed at the final reduce.
            picked = small.tile([P, PICK_F], _f32)
            for j in range(PICK_F):
                nc.gpsimd.indirect_dma_start(
                    out=picked[:, j : j + 1],
                    out_offset=None,
                    in_=y.ap(),
                    in_offset=bass.IndirectOffsetOnAxis(
                        ap=idx_sb[:, j : j + 1], axis=0
                    ),
                )
            # s_all[:, -1] = -sum_j picked[p, j]
            nc.vector.tensor_reduce(
                out=s_all[:, N_CHUNKS_DEV : N_CHUNKS_DEV + 1], in_=picked[:],
                axis=_AXF, op=_ADD, negate=True,
            )

            # stream the chunks through ACT in arrival order
            et = small.tile([P, FH], _bf16, tag="e")
            for c in EXP_ORDER:
                xt = x_tiles.pop(c)
                w = CHUNKS[c][2] * (F // 8)
                nc.scalar.activation(
                    out=et[:, 0:w], in_=xt[:, 0:w], func=_EXP,
                    bias=cbias[:, 0:1], scale=1.0,
                    accum_out=s_all[:, c : c + 1],
                )
                issue_dma(CHUNK_STREAM[c])

            # acc[j] = sum_p s_all[p, j]  (20 chunk sums + -picksum)
            acc = psum.tile([N_CHUNKS_DEV + 1, 1], _f32, tag="acc")
            nc.tensor.matmul(
                out=acc[:], lhsT=s_all[:], rhs=ones[:], start=True, stop=True
            )
            res = small.tile([N_CHUNKS_DEV + 1, 1], _f32)
            nc.vector.tensor_copy(out=res[:], in_=acc[:])
            nc.sync.dma_start(out=out.ap(), in_=res[:])

    nc.compile()
    return nc


def make_in_maps(y_hat: np.ndarray, coords: np.ndarray):
    """Shard inputs across cores and build per-core gather indices."""
    y_hat = np.ascontiguousarray(y_hat, dtype=np.float32)
    coords = np.asarray(coords, dtype=np.float32)

    # Match jnp.round (round-half-to-even); np.round has identical semantics,
    # and coords * 128 is exact in f32 (power-of-two scale).
    xi = np.round(coords[:, :, 0] * np.float32(G)).astype(np.int64)  # (B, T)
    yi = np.round(coords[:, :, 1] * np.float32(G)).astype(np.int64)  # (B, T)
    t = np.arange(T, dtype=np.int64)[None, :]
    flat = t * (G * G) + xi * G + yi  # (B, T) element offset within row b

    in_maps = []
    for c in range(N_CORES):
        rows = slice(c * ROWS, (c + 1) * ROWS)
        shard = y_hat[rows].reshape(N_PER_CORE, 1)
        local = np.arange(ROWS, dtype=np.int64)[:, None] * ROW_ELEMS + flat[rows]
        idx = local.reshape(P, PICK_F).astype(np.int32)
        in_maps.append({"y": shard, "idx": idx})
    return in_maps


def kernel(y_hat: np.ndarray, coords: np.ndarray) -> np.ndarray:
    global _compiled_nc, LAST_RESULTS
    in_maps = make_in_maps(y_hat, coords)
    if _compiled_nc is None:
        _compiled_nc = build_nc()
    res = run_bass_kernel_spmd(
        _compiled_nc, in_maps, core_ids=list(range(N_CORES))
    )
    LAST_RESULTS = res
    total = 0.0
    for r in res.results:
        v = np.asarray(r["out"]).reshape(-1).astype(np.float64)
        negpick = v[N_CHUNKS_DEV]
        s_rows = np.array([v[cols].sum() for cols in ROW_COLS])
        total += T * float(np.log(s_rows).sum()) + negpick
    loss = total / B + T * (-C_SHIFT)
    return np.array(np.float32(loss))


# revision 18
# speedup vs baseline: 1.1137x; 1.0057x over previous
"""Trainium2 Bass kernel for the CrossEntropyMap loss.

Math (per batch row b of y_hat[B=64, T=64, G=128, G]):
    lse_b  = logsumexp(y_hat[b].reshape(-1))            # over T*G*G = 1M classes
    pick_b = sum_t y_hat[b, t, xi[b,t], yi[b,t]]        # xi/yi = round(coords*G)
    loss   = mean_b(T * lse_b - pick_b)

Sharding: data-parallel over batch, 8 rows per NeuronCore (32 MiB/core).

Device kernel (per core): stream the 8 rows as 16 half-row [128, 4096] chunks
and run one ACT pass per chunk: exp(x + C_SHIFT) with accum_out giving the
per-partition sums S[p, c]. The constant shift is mathematically exact for
logsumexp (it only scales the partial sums); C_SHIFT=-16 keeps exp in range
for |x| up to ~100. The 512 picked logits are gathered with indirect DMAs
(f32, straight from HBM). One PE matmul with a ones vector reduces
[S | -picksum] over partitions to a [17, 1] output per core; the host folds
ln(), the shift and the batch mean while unsharding (64 scalar lns total).

DMA strategy: a single DGE queue only sustains ~210-240 GB/s, but with both
HWDGE rings (sync = qSPDynamicHW, scalar = qActDynamicHW) streaming
concurrently the 16 SDMA engines saturate at ~425 GB/s aggregate (measured;
~26.6 GB/s per SBUF AXI port). Chunks alternate between the two rings with
EQUAL bytes so both stay busy until the very end, and exps consume them in
the same alternating order so ACT tracks arrivals without head-of-line
blocking. A third SWDGE bulk stream or f32->bf16 in-flight cast does NOT
raise the ceiling (same 16 engines bind on the read side) — measured 425
GB/s either way — so gpsimd only runs the pick gather.
"""

import sys

import numpy as np

try:
    import concourse.bacc as bacc
except ImportError:  # pragma: no cover - fallback for bare environments
    sys.path.insert(0, "/opt/trn_rl_repo")
    import concourse.bacc as bacc

import concourse.bass as bass
import concourse.tile as tile
from concourse import mybir
from concourse.bass_utils import run_bass_kernel_spmd

B, T, G = 64, 64, 128
N_CORES = 8
ROWS = B // N_CORES            # 8 batch rows per core
ROW_ELEMS = T * G * G          # 1_048_576 classes per row
P = 128
F = ROW_ELEMS // P             # 8192 elements per partition per row
HALVES = 2                     # chunks per row
FH = F // HALVES               # 4096 per chunk
N_CHUNKS = ROWS * HALVES       # 16
N_PER_CORE = ROWS * ROW_ELEMS  # 8_388_608 elements per core shard
PICKS = ROWS * T               # 512 gathered logits per core
PICK_F = PICKS // P            # 4 per partition
C_SHIFT = -16.0                # constant exp bias (exact for logsumexp)

_f32 = mybir.dt.float32
_bf16 = mybir.dt.bfloat16
_i32 = mybir.dt.int32
_EXP = mybir.ActivationFunctionType.Exp
_AXF = mybir.AxisListType.X
_ADD = mybir.AluOpType.add

# --- stream configuration ---------------------------------------------------
# 'sy' = sync HWDGE ring, 'sc' = scalar HWDGE ring. Measured: the 16 SDMA
# engines cap at ~425 GB/s aggregate (~26.6 GB/s per port) once >=2
# descriptor streams are in flight, while a single stream only sustains
# ~210-240 GB/s — so split the bytes EQUALLY between both HWDGE rings and
# keep both busy until the very end. A third SWDGE bulk stream does not
# raise the ceiling (same engines), so gpsimd only runs the pick gather.
#
# Chunk list: (row, eighth_start, n_eighths) in units of F/8 = 1024 elems
# per partition. Row 0 tapers IN (2x 1 MiB then 2 MiB) so the first exp can
# start ~2.5us earlier — this matters in runs where ACT is p-state throttled
# (measured 1.0 GHz instead of 1.2 GHz) and becomes the critical path. Rows
# 1-6 stream as half-row 2 MiB chunks; row 7 tapers OUT (one 1 MiB chunk per
# ring at the end) so the two queues' simultaneous drain doesn't leave a
# serialized 2x3.7us exp tail. Descriptors below 8 KiB/partition measurably
# slow the stream, so the taper stops at 1 MiB chunks. Queue assignment is
# balanced to exactly 16 MiB per ring in near-alternating order.
CHUNKS = [(0, 0, 2), (0, 2, 2), (0, 4, 4)]
for _r in range(1, 7):
    CHUNKS += [(_r, 0, 4), (_r, 4, 4)]
CHUNKS += [(7, 0, 4), (7, 4, 2), (7, 6, 2)]
N_CHUNKS_DEV = len(CHUNKS)                                      # 18
CHUNK_STREAM = ["sy", "sc", "sy", "sc", "sy", "sc", "sy", "sc", "sy", "sc",
                "sy", "sc", "sy", "sc", "sy", "sc", "sy", "sc"]
# exp consumption order = arrival order (queues deliver alternately).
EXP_ORDER = list(range(N_CHUNKS_DEV))
# host mapping: per-row list of device columns to sum for that row's S
ROW_COLS = ([[0, 1, 2]] + [[2 * r + 1, 2 * r + 2] for r in range(1, 7)]
            + [[15, 16, 17]])
# deep prefill: 10 tiles in flight so most DMA dispatches run in the head
# (where ACT idles) instead of between exps on the ACT sequencer.
PREFILL = {"sy": 5, "sc": 5}

_compiled_nc = None
LAST_RESULTS = None  # test hook: BassKernelResults of the last run


def build_nc():
    nc = bacc.Bacc("TRN2", target_bir_lowering=False, debug=False)
    y = nc.dram_tensor("y", [N_PER_CORE, 1], _f32, kind="ExternalInput")
    idx = nc.dram_tensor("idx", [P, PICK_F], _i32, kind="ExternalInput")
    out = nc.dram_tensor("out", [N_CHUNKS_DEV + 1, 1], _f32, kind="ExternalOutput")

    # chunk views at half/quarter/eighth-row granularity: partition p of row r
    # holds elements [r*1M + p*8192, +8192) — contiguous per partition, so any
    # run of eighths (1024 elems) is one contiguous span per partition.
    y_view = {
        4: y.ap().rearrange("(r p h f) o -> r h p (f o)", r=ROWS, p=P, h=2),
        2: y.ap().rearrange("(r p q f) o -> r q p (f o)", r=ROWS, p=P, q=4),
        1: y.ap().rearrange("(r p e f) o -> r e p (f o)", r=ROWS, p=P, e=8),
    }

    def chunk_ap(c):
        r, e0, n = CHUNKS[c]
        assert e0 % n == 0
        return y_view[n][r, e0 // n]

    with tile.TileContext(nc) as tc:
        with (
            tc.tile_pool(name="xpool", bufs=sum(PREFILL.values())) as xpool,
            tc.tile_pool(name="small", bufs=1) as small,
            tc.tile_pool(name="psum", bufs=1, space="PSUM") as psum,
        ):
            engines = {"sy": nc.sync, "sc": nc.scalar}

            ones = small.tile([P, 1], _f32)
            nc.vector.memset(ones[:], 1.0)
            cbias = small.tile([P, 1], _f32)
            nc.vector.memset(cbias[:], C_SHIFT)
            idx_sb = small.tile([P, PICK_F], _i32)
            nc.sync.dma_start(out=idx_sb[:], in_=idx.ap())

            # s_all[:, c] = per-partition sum of exp(chunk c); last col = -picksum
            s_all = small.tile([P, N_CHUNKS_DEV + 1], _f32)

            # per-stream chunk lists in consumption order
            stream_chunks = {s: [c for c in EXP_ORDER if CHUNK_STREAM[c] == s]
                             for s in ("sy", "sc")}
            next_issue = {s: 0 for s in stream_chunks}
            x_tiles = {}

            def issue_dma(s):
                i = next_issue[s]
                if i >= len(stream_chunks[s]):
                    return
                next_issue[s] = i + 1
                c = stream_chunks[s][i]
                w = CHUNKS[c][2] * (F // 8)
                xt = xpool.tile([P, FH], _f32, tag="x")
                engines[s].dma_start(out=xt[:, 0:w], in_=chunk_ap(c))
                x_tiles[c] = xt

            # prefill both rings in global chunk order so buffer rotation
            # matches consumption order
            for c in range(PREFILL["sy"] + PREFILL["sc"]):
                issue_dma(CHUNK_STREAM[c])

            # picked-logit gather on the otherwise idle SWDGE queue; data is
            # only needed at the final reduce.
            picked = small.tile([P, PICK_F], _f32)
            for j in range(PICK_F):
                nc.gpsimd.indirect_dma_start(
                    out=picked[:, j : j + 1],
                    out_offset=None,
                    in_=y.ap(),
                    in_offset=bass.IndirectOffsetOnAxis(
                        ap=idx_sb[:, j : j + 1], axis=0
                    ),
                )
            # s_all[:, -1] = -sum_j picked[p, j]
            nc.vector.tensor_reduce(
                out=s_all[:, N_CHUNKS_DEV : N_CHUNKS_DEV + 1], in_=picked[:],
                axis=_AXF, op=_ADD, negate=True,
            )

            # stream the chunks through ACT in arrival order
            et = small.tile([P, FH], _bf16, tag="e")
            for c in EXP_ORDER:
                xt = x_tiles.pop(c)
                w = CHUNKS[c][2] * (F // 8)
                nc.scalar.activation(
                    out=et[:, 0:w], in_=xt[:, 0:w], func=_EXP,
                    bias=cbias[:, 0:1], scale=1.0,
                    accum_out=s_all[:, c : c + 1],
                )
                issue_dma(CHUNK_STREAM[c])

            # acc[j] = sum_p s_all[p, j]  (20 chunk sums + -picksum)
            acc = psum.tile([N_CHUNKS_DEV + 1, 1], _f32, tag="acc")
            nc.tensor.matmul(
                out=acc[:], lhsT=s_all[:], rhs=ones[:], start=True, stop=True
            )
            res = small.tile([N_CHUNKS_DEV + 1, 1], _f32)
            nc.vector.tensor_copy(out=res[:], in_=acc[:])
            nc.sync.dma_start(out=out.ap(), in_=res[:])

    nc.compile()
    return nc


def make_in_maps(y_hat: np.ndarray, coords: np.ndarray):
    """Shard inputs across cores and build per-core gather indices."""
    y_hat = np.ascontiguousarray(y_hat, dtype=np.float32)
    coords = np.asarray(coords, dtype=np.float32)

    # Match jnp.round (round-half-to-even); np.round has identical semantics,
    # and coords * 128 is exact in f32 (power-of-two scale).
    xi = np.round(coords[:, :, 0] * np.float32(G)).astype(np.int64)  # (B, T)
    yi = np.round(coords[:, :, 1] * np.float32(G)).astype(np.int64)  # (B, T)
    t = np.arange(T, dtype=np.int64)[None, :]
    flat = t * (G * G) + xi * G + yi  # (B, T) element offset within row b

    in_maps = []
    for c in range(N_CORES):
        rows = slice(c * ROWS, (c + 1) * ROWS)
        shard = y_hat[rows].reshape(N_PER_CORE, 1)
        local = np.arange(ROWS, dtype=np.int64)[:, None] * ROW_ELEMS + flat[rows]
        idx = local.reshape(P, PICK_F).astype(np.int32)
        in_maps.append({"y": shard, "idx": idx})
    return in_maps


def kernel(y_hat: np.ndarray, coords: np.ndarray) -> np.ndarray:
    global _compiled_nc, LAST_RESULTS
    in_maps = make_in_maps(y_hat, coords)
    if _compiled_nc is None:
        _compiled_nc = build_nc()
    res = run_bass_kernel_spmd(
        _compiled_nc, in_maps, core_ids=list(range(N_CORES))
    )
    LAST_RESULTS = res
    total = 0.0
    for r in res.results:
        v = np.asarray(r["out"]).reshape(-1).astype(np.float64)
        negpick = v[N_CHUNKS_DEV]
        s_rows = np.array([v[cols].sum() for cols in ROW_COLS])
        total += T * float(np.log(s_rows).sum()) + negpick
    loss = total / B + T * (-C_SHIFT)
    return np.array(np.float32(loss))
